# revision 10
# baseline (speedup 1.0000x reference)
"""Trainium2 Bass kernel for GQA attention (dense_transformer).

Module: B=1, S=2048, D=2048, 32 q heads / 8 kv heads, head_dim 64,
llama3-scaled interleaved RoPE, causal + segment mask, softmax, out proj.

Sharding (per the mp hint): 8 cores, core c owns q heads [4c..4c+3] and
kv head c.  Each core computes its 4 heads' attention output and a
partial output projection (wo rows [256c..256c+256)); the host sums the
8 partial projections (the mp "all-reduce" of row-parallel wo).

Device dataflow per core (all matmuls fp32r on the PE at 1 cyc/row):
  - qkv proj: lhsT = X^T chunks [128d,128s] (streamed), rhs = [wq|wk|wv]
    shard [128d, 384] -> psum [128 s, 384] per s-block.
  - RoPE applied in natural [s, hd] layout on DVE straight out of PSUM,
    then PE-transposed to qT [256, S] / kT [64, S] (dh on partitions).
  - v evicted natural [s, 64] + ones column -> v_aug [128, 65] per block.
  - scoresT[k,q] blocks: lhsT=kT block, rhs=qT slice; exp on ScalarE
    (scale=1/8 folded) psum->sbuf; causal mask via gpsimd.affine_select
    on diagonal blocks only; blocks above the diagonal are skipped.
  - PV: psum_o[65, 512] += v_aug.T @ p  (ones row accumulates the
    softmax denominator); normalize with DVE after partition_broadcast
    of the reciprocal denominator row.
  - out proj: out[s-block, :] partial = AT.T @ wo_shard.
"""

import math
from contextlib import ExitStack

import numpy as np

import concourse.bass as bass
import concourse.mybir as mybir
import concourse.tile as tile
from concourse import bacc
from concourse.masks import make_identity
from concourse.bass_utils import run_bass_kernel_spmd

F32 = mybir.dt.float32
F32R = mybir.dt.float32r
EXP = mybir.ActivationFunctionType.Exp

HEAD_DIM = 64
N_HEADS = 32
N_KV = 8
THETA = 500000.0
MAX_POS = 4096
N_CORES = 8

S = 2048
D = 2048
HPC = N_HEADS // N_CORES      # q heads per core = 4
QW = HPC * HEAD_DIM           # q cols per core = 256
SB = S // 128                 # 16 s-blocks of 128
DB = D // 128                 # 16 d-slices of 128
QT = S // 512                 # 4 q-tiles of 512
KB = S // 128                 # 16 k-blocks of 128


def _rope_tables(head_dim, end, theta, use_scaled=True):
    freqs = 1.0 / (theta ** (np.arange(0, head_dim, 2, dtype=np.float32) / head_dim))
    if use_scaled:  # llama3 rope scaling
        scale_factor, low_f, high_f, old_ctx = 8.0, 1.0, 4.0, 8192.0
        low_wl, high_wl = old_ctx / low_f, old_ctx / high_f
        wavelen = 2.0 * np.pi / freqs
        smooth = (old_ctx / wavelen - low_f) / (high_f - low_f)
        freqs = np.where(
            wavelen < high_wl, freqs,
            np.where(wavelen > low_wl, freqs / scale_factor,
                     (1.0 - smooth) * freqs / scale_factor + smooth * freqs),
        ).astype(np.float32)
    t = np.arange(end, dtype=np.float32)
    ang = np.outer(t, freqs)  # [end, head_dim//2]
    return np.cos(ang).astype(np.float32), np.sin(ang).astype(np.float32)


def build_kernel():
    # Bacc (not plain Bass): its compile() pass pipeline legalizes multi-wait
    # instructions (TRN2 allows at most 1 sync wait per instruction).
    nc = bacc.Bacc()
    xt = nc.declare_dram_parameter("xt", [D, S], F32R, isOutput=False)
    w_all = nc.declare_dram_parameter("w_all", [D, QW + 2 * HEAD_DIM], F32R, isOutput=False)
    wo = nc.declare_dram_parameter("wo", [QW, D], F32R, isOutput=False)
    cos_d = nc.declare_dram_parameter("cos", [S, HEAD_DIM // 2], F32, isOutput=False)
    sin_d = nc.declare_dram_parameter("sin", [S, HEAD_DIM // 2], F32, isOutput=False)
    out_d = nc.declare_dram_parameter("out", [S, D], F32, isOutput=True)

    with ExitStack() as ctx:
        tc = ctx.enter_context(tile.TileContext(nc))

        # ---- persistent sbuf tensors (bufs=1, distinct tags) ----
        persist = ctx.enter_context(tc.tile_pool(name="persist", bufs=1))
        w_sb = [persist.tile([128, 384], F32R, tag=f"w{d}", name=f"w{d}") for d in range(DB)]
        cos_sb = persist.tile([128, SB * 32], F32, tag="cos", name="cos_sb")   # block m at cols 32m
        sin_sb = persist.tile([128, SB * 32], F32, tag="sin", name="sin_sb")
        qT_sb = [persist.tile([128, S], F32R, tag=f"qT{t}", name=f"qT{t}") for t in range(2)]
        # kT duplicated into both partition halves so scores lhsT can match
        # the q head's partition base (matmul requires equal base partitions)
        kT_sb = persist.tile([128, S], F32R, tag="kT", name="kT")
        v_aug = [persist.tile([128, HEAD_DIM + 1], F32R, tag=f"vaug{m}", name=f"vaug{m}") for m in range(SB)]
        AT_sb = [persist.tile([128, S], F32R, tag=f"AT{t}", name=f"AT{t}") for t in range(2)]
        wo_sb = [persist.tile([128, D], F32R, tag=f"wo{t}", name=f"wo{t}") for t in range(2)]
        ident = persist.tile([128, 128], F32, tag="ident")

        # ---- rotating pools ----
        xt_pool = ctx.enter_context(tc.tile_pool(name="xt", bufs=8))
        rot_pool = ctx.enter_context(tc.tile_pool(name="rot", bufs=3))
        tmp_pool = ctx.enter_context(tc.tile_pool(name="tmp", bufs=4))
        p_pool = ctx.enter_context(tc.tile_pool(name="p", bufs=4))
        o_sb_pool = ctx.enter_context(tc.tile_pool(name="osb", bufs=3))
        nrm_pool = ctx.enter_context(tc.tile_pool(name="nrm", bufs=3))
        ps_mm = ctx.enter_context(tc.tile_pool(name="psmm", bufs=4, space="PSUM"))
        ps_o = ctx.enter_context(tc.tile_pool(name="pso", bufs=2, space="PSUM"))

        # ---- setup: weights, tables, identity, ones columns ----
        make_identity(nc, ident)
        for d in range(DB):
            nc.sync.dma_start(out=w_sb[d], in_=w_all[d * 128:(d + 1) * 128, :])
        # cos/sin: DRAM [S, 32] -> sbuf [128, (m f)] ; partition p = s % 128
        cos_r = cos_d.rearrange("(m p) f -> p m f", p=128)
        sin_r = sin_d.rearrange("(m p) f -> p m f", p=128)
        nc.sync.dma_start(out=cos_sb.rearrange("p (m f) -> p m f", m=SB), in_=cos_r)
        nc.sync.dma_start(out=sin_sb.rearrange("p (m f) -> p m f", m=SB), in_=sin_r)
        for t in range(2):
            nc.sync.dma_start(out=wo_sb[t], in_=wo[t * 128:(t + 1) * 128, :])
        ones_col = persist.tile([128, 1], F32, tag="ones", name="ones_col")
        nc.vector.memset(ones_col, 1.0)
        for m in range(SB):
            # memset can't write f32r; copy-cast from an f32 ones tile
            nc.vector.tensor_copy(v_aug[m][:, HEAD_DIM:HEAD_DIM + 1], ones_col)

        # ---- phase B: qkv projection + rope + transposes ----
        for m in range(SB):
            ps_qkv = ps_mm.tile([128, 512], F32, tag="mm")
            for d in range(DB):
                xc = xt_pool.tile([128, 128], F32R)
                nc.sync.dma_start(
                    out=xc, in_=xt[d * 128:(d + 1) * 128, m * 128:(m + 1) * 128])
                nc.tensor.matmul(
                    ps_qkv[:, 0:384],
                    lhsT=xc,
                    rhs=w_sb[d],
                    start=(d == 0), stop=(d == DB - 1))

            cm = cos_sb[:, m * 32:(m + 1) * 32]
            sm = sin_sb[:, m * 32:(m + 1) * 32]
            cmq = cm[:, None, :].broadcast_to([128, HPC, 32])
            smq = sm[:, None, :].broadcast_to([128, HPC, 32])

            # q rope: psum cols 0..255 viewed as [h, f, 2]
            q4 = ps_qkv[:, 0:QW].rearrange("p (h f t) -> p h f t", h=HPC, t=2)
            q_re, q_im = q4[:, :, :, 0], q4[:, :, :, 1]
            qrot = rot_pool.tile([128, QW], F32, tag="qrot")
            qr4 = qrot.rearrange("p (h f t) -> p h f t", h=HPC, t=2)
            e1 = tmp_pool.tile([128, HPC, 32], F32, tag="t1")
            e2 = tmp_pool.tile([128, HPC, 32], F32, tag="t2")
            nc.vector.tensor_mul(e1, q_re, cmq)
            nc.vector.tensor_mul(e2, q_im, smq)
            nc.vector.tensor_sub(qr4[:, :, :, 0], e1, e2)
            o1 = tmp_pool.tile([128, HPC, 32], F32, tag="t3")
            o2 = tmp_pool.tile([128, HPC, 32], F32, tag="t4")
            nc.vector.tensor_mul(o1, q_re, smq)
            nc.vector.tensor_mul(o2, q_im, cmq)
            nc.vector.tensor_add(qr4[:, :, :, 1], o1, o2)

            # k rope: psum cols 256..319 viewed as [f, 2]
            k4 = ps_qkv[:, QW:QW + HEAD_DIM].rearrange("p (f t) -> p f t", t=2)
            krot = rot_pool.tile([128, HEAD_DIM], F32, tag="krot")
            kr4 = krot.rearrange("p (f t) -> p f t", t=2)
            f1 = tmp_pool.tile([128, 32], F32, tag="t5")
            f2 = tmp_pool.tile([128, 32], F32, tag="t6")
            nc.vector.tensor_mul(f1, k4[:, :, 0], cm)
            nc.vector.tensor_mul(f2, k4[:, :, 1], sm)
            nc.vector.tensor_sub(kr4[:, :, 0], f1, f2)
            g1 = tmp_pool.tile([128, 32], F32, tag="t7")
            g2 = tmp_pool.tile([128, 32], F32, tag="t8")
            nc.vector.tensor_mul(g1, k4[:, :, 0], sm)
            nc.vector.tensor_mul(g2, k4[:, :, 1], cm)
            nc.vector.tensor_add(kr4[:, :, 1], g1, g2)

            # v: straight eviction (no rope), natural [s, 64]
            nc.vector.tensor_copy(
                v_aug[m][:, 0:HEAD_DIM], ps_qkv[:, QW + HEAD_DIM:QW + 2 * HEAD_DIM])

            # transposes -> qT, kT
            for t in range(2):
                pst = ps_mm.tile([128, 512], F32, tag="mm")
                nc.tensor.transpose(
                    pst[:, 0:128], qrot[:, t * 128:(t + 1) * 128], ident)
                nc.vector.tensor_copy(
                    qT_sb[t][:, m * 128:(m + 1) * 128], pst[:, 0:128])
            psk = ps_mm.tile([128, 512], F32, tag="mm")
            nc.tensor.transpose(psk[0:64, 0:128], krot, ident)
            nc.vector.tensor_copy(
                kT_sb[0:64, m * 128:(m + 1) * 128], psk[0:64, 0:128])
            nc.vector.tensor_copy(
                kT_sb[64:128, m * 128:(m + 1) * 128], psk[0:64, 0:128])

        # ---- phase C: attention (j outer for early out-proj), phase D folded in ----
        for j in range(QT):
            for h in range(HPC):
                pso = ps_o.tile([HEAD_DIM + 1, 512], F32, tag="o")
                ilast = 4 * j + 3
                qslice = qT_sb[h // 2][64 * (h % 2):64 * (h % 2) + 64,
                                      j * 512:(j + 1) * 512]
                for i in range(4 * j + 4):
                    pss = ps_mm.tile([128, 512], F32, tag="mm")
                    nc.tensor.matmul(
                        pss,
                        lhsT=kT_sb[64 * (h % 2):64 * (h % 2) + 64,
                                   i * 128:(i + 1) * 128],
                        rhs=qslice,
                        start=True, stop=True)
                    p = p_pool.tile([128, 512], F32R, tag="p")
                    nc.scalar.activation(p, pss, EXP, scale=0.125)
                    if i >= 4 * j:  # diagonal block: zero out k > q
                        nc.gpsimd.affine_select(
                            out=p, in_=p,
                            compare_op=mybir.AluOpType.is_ge,
                            fill=0.0,
                            base=512 * j - 128 * i,
                            channel_multiplier=-1,
                            pattern=[[1, 512]])
                    nc.tensor.matmul(
                        pso,
                        lhsT=v_aug[i],
                        rhs=p,
                        start=(i == 0), stop=(i == ilast))
                rd = nrm_pool.tile([1, 512], F32, tag="rd")
                nc.vector.reciprocal(rd, pso[HEAD_DIM:HEAD_DIM + 1, :])
                rdb = nrm_pool.tile([64, 512], F32, tag="rdb")
                nc.gpsimd.partition_broadcast(rdb, rd)
                nc.vector.tensor_mul(
                    AT_sb[h // 2][64 * (h % 2):64 * (h % 2) + 64,
                                  j * 512:(j + 1) * 512],
                    pso[0:HEAD_DIM, :], rdb)

            # ---- phase D for the s-blocks covered by this q-tile ----
            for m in range(4 * j, 4 * j + 4):
                out_sb = o_sb_pool.tile([128, D], F32, tag="out")
                for n in range(4):
                    psd = ps_mm.tile([128, 512], F32, tag="mm")
                    for t in range(2):
                        nc.tensor.matmul(
                            psd,
                            lhsT=AT_sb[t][:, m * 128:(m + 1) * 128],
                            rhs=wo_sb[t][:, n * 512:(n + 1) * 512],
                            start=(t == 0), stop=(t == 1))
                    if n % 2 == 0:
                        nc.scalar.copy(out_sb[:, n * 512:(n + 1) * 512], psd)
                    else:
                        nc.vector.tensor_copy(out_sb[:, n * 512:(n + 1) * 512], psd)
                nc.sync.dma_start(
                    out=out_d[m * 128:(m + 1) * 128, :], in_=out_sb)

    nc.finalize()
    return nc


_NC_CACHE = None


def _get_nc():
    global _NC_CACHE
    if _NC_CACHE is None:
        _NC_CACHE = build_kernel()
    return _NC_CACHE


def kernel(hidden_states, attention_mask, position_ids, wq, wk, wv, wo,
           _trace=False, _trace_kwargs=None):
    hidden_states = np.asarray(hidden_states)
    attention_mask = np.asarray(attention_mask)
    position_ids = np.asarray(position_ids)
    wq, wk, wv, wo = (np.asarray(a) for a in (wq, wk, wv, wo))
    B, S_, D_ = hidden_states.shape
    assert (B, S_, D_) == (1, S, D)
    out_dtype = hidden_states.dtype

    seg = attention_mask[0]
    if not np.all(seg == seg[0]):
        raise NotImplementedError(
            "non-uniform attention_mask (multi-segment) not supported")

    cos_t, sin_t = _rope_tables(HEAD_DIM, MAX_POS, THETA, use_scaled=True)
    pos = position_ids[0].astype(np.int64)
    cos = np.ascontiguousarray(cos_t[pos])  # [S, 32]
    sin = np.ascontiguousarray(sin_t[pos])

    x = np.ascontiguousarray(hidden_states[0].astype(np.float32))
    xt = np.ascontiguousarray(x.T)  # [D, S]

    in_maps = []
    for c in range(N_CORES):
        w_cat = np.ascontiguousarray(np.concatenate([
            wq[:, c * QW:(c + 1) * QW],
            wk[:, c * HEAD_DIM:(c + 1) * HEAD_DIM],
            wv[:, c * HEAD_DIM:(c + 1) * HEAD_DIM],
        ], axis=1).astype(np.float32))
        wo_rows = np.ascontiguousarray(
            wo[c * QW:(c + 1) * QW, :].astype(np.float32))
        in_maps.append({
            "xt": xt, "w_all": w_cat, "wo": wo_rows, "cos": cos, "sin": sin,
        })

    nc = _get_nc()
    kw = {}
    if _trace:
        kw["trace"] = True
        if _trace_kwargs:
            kw.update(_trace_kwargs)
    res = run_bass_kernel_spmd(nc, in_maps, list(range(N_CORES)), **kw)
    out = np.zeros((S, D), np.float32)
    for c in range(N_CORES):
        out += res.results[c]["out"]
    kernel._last_result = res
    return out.reshape(1, S, D).astype(out_dtype)


# revision 14
# speedup vs baseline: 1.2637x; 1.2637x over previous
"""Trainium2 Bass kernel for GQA attention (dense_transformer).

Module: B=1, S=2048, D=2048, 32 q heads / 8 kv heads, head_dim 64,
llama3-scaled interleaved RoPE, causal + segment mask, softmax, out proj.

Sharding (per the mp hint): 8 cores, core c owns q heads [4c..4c+3] and
kv head c.  Each core computes its 4 heads' attention output and a
partial output projection (wo rows [256c..256c+256)); the host sums the
8 partial projections (the mp "all-reduce" of row-parallel wo).

Device dataflow per core (all matmuls fp32r on the PE at 1 cyc/row):
  - qkv proj: lhsT = X^T chunks [128d,128s] (streamed), rhs = [wq|wk|wv]
    shard [128d, 384] -> psum [128 s, 384] per s-block.
  - RoPE applied in natural [s, hd] layout on DVE straight out of PSUM,
    then PE-transposed to qT [256, S] / kT [64, S] (dh on partitions).
  - v evicted natural [s, 64] + ones column -> v_aug [128, 65] per block.
  - scoresT[k,q] blocks: lhsT=kT block, rhs=qT slice; exp on ScalarE
    (scale=1/8 folded) psum->sbuf; causal mask via gpsimd.affine_select
    on diagonal blocks only; blocks above the diagonal are skipped.
  - PV: psum_o[65, 512] += v_aug.T @ p  (ones row accumulates the
    softmax denominator); normalize with DVE after partition_broadcast
    of the reciprocal denominator row.
  - out proj: out[s-block, :] partial = AT.T @ wo_shard.
"""

import math
from contextlib import ExitStack

import numpy as np

import concourse.bass as bass
import concourse.mybir as mybir
import concourse.tile as tile
from concourse import bacc
from concourse.masks import make_identity
from concourse.bass_utils import run_bass_kernel_spmd

F32 = mybir.dt.float32
F32R = mybir.dt.float32r
EXP = mybir.ActivationFunctionType.Exp

HEAD_DIM = 64
N_HEADS = 32
N_KV = 8
THETA = 500000.0
MAX_POS = 4096
N_CORES = 8

S = 2048
D = 2048
HPC = N_HEADS // N_CORES      # q heads per core = 4
QW = HPC * HEAD_DIM           # q cols per core = 256
SB = S // 128                 # 16 s-blocks of 128
DB = D // 128                 # 16 d-slices of 128
QT = S // 512                 # 4 q-tiles of 512
KB = S // 128                 # 16 k-blocks of 128


def _rope_tables(head_dim, end, theta, use_scaled=True):
    freqs = 1.0 / (theta ** (np.arange(0, head_dim, 2, dtype=np.float32) / head_dim))
    if use_scaled:  # llama3 rope scaling
        scale_factor, low_f, high_f, old_ctx = 8.0, 1.0, 4.0, 8192.0
        low_wl, high_wl = old_ctx / low_f, old_ctx / high_f
        wavelen = 2.0 * np.pi / freqs
        smooth = (old_ctx / wavelen - low_f) / (high_f - low_f)
        freqs = np.where(
            wavelen < high_wl, freqs,
            np.where(wavelen > low_wl, freqs / scale_factor,
                     (1.0 - smooth) * freqs / scale_factor + smooth * freqs),
        ).astype(np.float32)
    t = np.arange(end, dtype=np.float32)
    ang = np.outer(t, freqs)  # [end, head_dim//2]
    return np.cos(ang).astype(np.float32), np.sin(ang).astype(np.float32)


def build_kernel():
    # Bacc (not plain Bass): its compile() pass pipeline legalizes multi-wait
    # instructions (TRN2 allows at most 1 sync wait per instruction).
    nc = bacc.Bacc()
    xt = nc.declare_dram_parameter("xt", [D, S], F32R, isOutput=False)
    w_all = nc.declare_dram_parameter("w_all", [D, QW + 2 * HEAD_DIM], F32R, isOutput=False)
    wo = nc.declare_dram_parameter("wo", [QW, D], F32R, isOutput=False)
    cos_d = nc.declare_dram_parameter("cos", [S, HEAD_DIM // 2], F32, isOutput=False)
    sin_d = nc.declare_dram_parameter("sin", [S, HEAD_DIM // 2], F32, isOutput=False)
    out_d = nc.declare_dram_parameter("out", [S, D], F32, isOutput=True)

    with ExitStack() as ctx:
        tc = ctx.enter_context(tile.TileContext(nc))

        # ---- persistent sbuf tensors (bufs=1, distinct tags) ----
        persist = ctx.enter_context(tc.tile_pool(name="persist", bufs=1))
        w_sb = [persist.tile([128, 384], F32R, tag=f"w{d}", name=f"w{d}") for d in range(DB)]
        cos_sb = persist.tile([128, SB * 32], F32, tag="cos", name="cos_sb")   # block m at cols 32m
        sin_sb = persist.tile([128, SB * 32], F32, tag="sin", name="sin_sb")
        qT_sb = [persist.tile([128, S], F32R, tag=f"qT{t}", name=f"qT{t}") for t in range(2)]
        # kT duplicated into both partition halves so scores lhsT can match
        # the q head's partition base (matmul requires equal base partitions)
        kT_sb = persist.tile([128, S], F32R, tag="kT", name="kT")
        v_aug = [persist.tile([128, HEAD_DIM + 1], F32R, tag=f"vaug{m}", name=f"vaug{m}") for m in range(SB)]
        AT_sb = [persist.tile([128, S], F32R, tag=f"AT{t}", name=f"AT{t}") for t in range(2)]
        wo_sb = [persist.tile([128, D], F32R, tag=f"wo{t}", name=f"wo{t}") for t in range(2)]
        ident = persist.tile([128, 128], F32, tag="ident")

        # ---- rotating pools ----
        xt_pool = ctx.enter_context(tc.tile_pool(name="xt", bufs=2))
        rot_pool = ctx.enter_context(tc.tile_pool(name="rot", bufs=3))
        tmp_pool = ctx.enter_context(tc.tile_pool(name="tmp", bufs=4))
        p_pool = ctx.enter_context(tc.tile_pool(name="p", bufs=4))
        o_sb_pool = ctx.enter_context(tc.tile_pool(name="osb", bufs=2))
        nrm_pool = ctx.enter_context(tc.tile_pool(name="nrm", bufs=3))
        ps_mm = ctx.enter_context(tc.tile_pool(name="psmm", bufs=2, space="PSUM"))
        ps_s = ctx.enter_context(tc.tile_pool(name="pss", bufs=2, space="PSUM"))
        ps_o = ctx.enter_context(tc.tile_pool(name="pso", bufs=2, space="PSUM"))

        # ---- setup: weights, tables, identity, ones columns ----
        make_identity(nc, ident)
        for d in range(DB):
            nc.sync.dma_start(out=w_sb[d], in_=w_all[d * 128:(d + 1) * 128, :])
        # cos/sin: DRAM [S, 32] -> sbuf [128, (m f)] ; partition p = s % 128
        cos_r = cos_d.rearrange("(m p) f -> p m f", p=128)
        sin_r = sin_d.rearrange("(m p) f -> p m f", p=128)
        nc.sync.dma_start(out=cos_sb.rearrange("p (m f) -> p m f", m=SB), in_=cos_r)
        nc.sync.dma_start(out=sin_sb.rearrange("p (m f) -> p m f", m=SB), in_=sin_r)
        for t in range(2):
            nc.sync.dma_start(out=wo_sb[t], in_=wo[t * 128:(t + 1) * 128, :])
        ones_col = persist.tile([128, 1], F32, tag="ones", name="ones_col")
        nc.vector.memset(ones_col, 1.0)
        for m in range(SB):
            # memset can't write f32r; copy-cast from an f32 ones tile
            nc.vector.tensor_copy(v_aug[m][:, HEAD_DIM:HEAD_DIM + 1], ones_col)

        # ---- phase B: qkv projection + rope + transposes ----
        xt_r = xt.rearrange("(db p) s -> p db s", p=128)
        for m in range(SB):
            # one batched DMA for the whole X^T column block of this s-tile
            xcol = xt_pool.tile([128, DB * 128], F32R, tag="xcol")
            nc.sync.dma_start(
                out=xcol.rearrange("p (db c) -> p db c", db=DB),
                in_=xt_r[:, :, m * 128:(m + 1) * 128])
            ps_qkv = ps_mm.tile([128, 512], F32, tag="mm")
            for d in range(DB):
                nc.tensor.matmul(
                    ps_qkv[:, 0:384],
                    lhsT=xcol[:, d * 128:(d + 1) * 128],
                    rhs=w_sb[d],
                    start=(d == 0), stop=(d == DB - 1))

            cm = cos_sb[:, m * 32:(m + 1) * 32]
            sm = sin_sb[:, m * 32:(m + 1) * 32]
            # fused rope over q (4 heads) and k (1 head): cols 0..319 have a
            # uniform [5, 32, 2] (group, freq, re/im) structure
            cm5 = cm[:, None, :].broadcast_to([128, HPC + 1, 32])
            sm5 = sm[:, None, :].broadcast_to([128, HPC + 1, 32])
            qk5 = ps_qkv[:, 0:QW + HEAD_DIM].rearrange(
                "p (g f t) -> p g f t", g=HPC + 1, t=2)
            qk_re, qk_im = qk5[:, :, :, 0], qk5[:, :, :, 1]
            qkrot = rot_pool.tile([128, QW + HEAD_DIM], F32, tag="qkrot")
            qkr5 = qkrot.rearrange("p (g f t) -> p g f t", g=HPC + 1, t=2)
            e1 = tmp_pool.tile([128, HPC + 1, 32], F32, tag="t1")
            e2 = tmp_pool.tile([128, HPC + 1, 32], F32, tag="t2")
            nc.vector.tensor_mul(e1, qk_re, cm5)
            nc.vector.tensor_mul(e2, qk_im, sm5)
            nc.vector.tensor_sub(qkr5[:, :, :, 0], e1, e2)
            o1 = tmp_pool.tile([128, HPC + 1, 32], F32, tag="t3")
            o2 = tmp_pool.tile([128, HPC + 1, 32], F32, tag="t4")
            nc.vector.tensor_mul(o1, qk_re, sm5)
            nc.vector.tensor_mul(o2, qk_im, cm5)
            nc.vector.tensor_add(qkr5[:, :, :, 1], o1, o2)

            # v: straight eviction (no rope), natural [s, 64]
            nc.vector.tensor_copy(
                v_aug[m][:, 0:HEAD_DIM], ps_qkv[:, QW + HEAD_DIM:QW + 2 * HEAD_DIM])

            # transposes -> qT, kT
            for t in range(2):
                pst = ps_mm.tile([128, 512], F32, tag="mm")
                nc.tensor.transpose(
                    pst[:, 0:128], qkrot[:, t * 128:(t + 1) * 128], ident)
                nc.vector.tensor_copy(
                    qT_sb[t][:, m * 128:(m + 1) * 128], pst[:, 0:128])
            psk = ps_mm.tile([128, 512], F32, tag="mm")
            nc.tensor.transpose(psk[0:64, 0:128], qkrot[:, QW:QW + HEAD_DIM], ident)
            nc.vector.tensor_copy(
                kT_sb[0:64, m * 128:(m + 1) * 128], psk[0:64, 0:128])
            nc.vector.tensor_copy(
                kT_sb[64:128, m * 128:(m + 1) * 128], psk[0:64, 0:128])

        # ---- phase C: attention (j outer for early out-proj), phase D folded in ----
        for j in range(QT):
            for h in range(HPC):
                pso = ps_o.tile([HEAD_DIM + 1, 512], F32, tag="o")
                ilast = 4 * j + 3
                qslice = qT_sb[h // 2][64 * (h % 2):64 * (h % 2) + 64,
                                      j * 512:(j + 1) * 512]
                for i0 in range(0, 4 * j + 4, 2):
                    # two k-blocks share one 2-bank psum tile so the exp
                    # (and diagonal mask) runs once per 1024 columns,
                    # amortizing the fixed per-instruction overhead
                    pss = ps_s.tile([128, 1024], F32, tag="s")
                    for u in range(2):
                        nc.tensor.matmul(
                            pss[:, u * 512:(u + 1) * 512],
                            lhsT=kT_sb[64 * (h % 2):64 * (h % 2) + 64,
                                       (i0 + u) * 128:(i0 + u) * 128 + 128],
                            rhs=qslice,
                            start=True, stop=True)
                    p = p_pool.tile([128, 1024], F32R, tag="p")
                    nc.scalar.activation(p, pss, EXP, scale=0.125)
                    for u in range(2):
                        if i0 + u >= 4 * j:  # diagonal block: zero out k > q
                            nc.gpsimd.affine_select(
                                out=p[:, u * 512:(u + 1) * 512],
                                in_=p[:, u * 512:(u + 1) * 512],
                                compare_op=mybir.AluOpType.is_ge,
                                fill=0.0,
                                base=512 * j - 128 * (i0 + u),
                                channel_multiplier=-1,
                                pattern=[[1, 512]])
                    for u in range(2):
                        nc.tensor.matmul(
                            pso,
                            lhsT=v_aug[i0 + u],
                            rhs=p[:, u * 512:(u + 1) * 512],
                            start=(i0 + u == 0), stop=(i0 + u == ilast))
                rd = nrm_pool.tile([1, 512], F32, tag="rd")
                nc.vector.reciprocal(rd, pso[HEAD_DIM:HEAD_DIM + 1, :])
                rdb = nrm_pool.tile([64, 512], F32, tag="rdb")
                nc.gpsimd.partition_broadcast(rdb, rd)
                nc.vector.tensor_mul(
                    AT_sb[h // 2][64 * (h % 2):64 * (h % 2) + 64,
                                  j * 512:(j + 1) * 512],
                    pso[0:HEAD_DIM, :], rdb)

            # ---- phase D for the s-blocks covered by this q-tile ----
            for m in range(4 * j, 4 * j + 4):
                out_sb = o_sb_pool.tile([128, D], F32, tag="out")
                for n in range(4):
                    psd = ps_mm.tile([128, 512], F32, tag="mm")
                    for t in range(2):
                        nc.tensor.matmul(
                            psd,
                            lhsT=AT_sb[t][:, m * 128:(m + 1) * 128],
                            rhs=wo_sb[t][:, n * 512:(n + 1) * 512],
                            start=(t == 0), stop=(t == 1))
                    nc.vector.tensor_copy(out_sb[:, n * 512:(n + 1) * 512], psd)
                nc.sync.dma_start(
                    out=out_d[m * 128:(m + 1) * 128, :], in_=out_sb)

    nc.finalize()
    return nc


_NC_CACHE = None


def _get_nc():
    global _NC_CACHE
    if _NC_CACHE is None:
        _NC_CACHE = build_kernel()
    return _NC_CACHE


def kernel(hidden_states, attention_mask, position_ids, wq, wk, wv, wo,
           _trace=False, _trace_kwargs=None):
    hidden_states = np.asarray(hidden_states)
    attention_mask = np.asarray(attention_mask)
    position_ids = np.asarray(position_ids)
    wq, wk, wv, wo = (np.asarray(a) for a in (wq, wk, wv, wo))
    B, S_, D_ = hidden_states.shape
    assert (B, S_, D_) == (1, S, D)
    out_dtype = hidden_states.dtype

    seg = attention_mask[0]
    if not np.all(seg == seg[0]):
        raise NotImplementedError(
            "non-uniform attention_mask (multi-segment) not supported")

    cos_t, sin_t = _rope_tables(HEAD_DIM, MAX_POS, THETA, use_scaled=True)
    pos = position_ids[0].astype(np.int64)
    cos = np.ascontiguousarray(cos_t[pos])  # [S, 32]
    sin = np.ascontiguousarray(sin_t[pos])

    x = np.ascontiguousarray(hidden_states[0].astype(np.float32))
    xt = np.ascontiguousarray(x.T)  # [D, S]

    in_maps = []
    for c in range(N_CORES):
        w_cat = np.ascontiguousarray(np.concatenate([
            wq[:, c * QW:(c + 1) * QW],
            wk[:, c * HEAD_DIM:(c + 1) * HEAD_DIM],
            wv[:, c * HEAD_DIM:(c + 1) * HEAD_DIM],
        ], axis=1).astype(np.float32))
        wo_rows = np.ascontiguousarray(
            wo[c * QW:(c + 1) * QW, :].astype(np.float32))
        in_maps.append({
            "xt": xt, "w_all": w_cat, "wo": wo_rows, "cos": cos, "sin": sin,
        })

    nc = _get_nc()
    kw = {}
    if _trace:
        kw["trace"] = True
        if _trace_kwargs:
            kw.update(_trace_kwargs)
    res = run_bass_kernel_spmd(nc, in_maps, list(range(N_CORES)), **kw)
    out = np.zeros((S, D), np.float32)
    for c in range(N_CORES):
        out += res.results[c]["out"]
    kernel._last_result = res
    return out.reshape(1, S, D).astype(out_dtype)


# revision 21
# speedup vs baseline: 1.4403x; 1.1398x over previous
"""Trainium2 Bass kernel for GQA attention (dense_transformer).

Module: B=1, S=2048, D=2048, 32 q heads / 8 kv heads, head_dim 64,
llama3-scaled interleaved RoPE, causal + segment mask, softmax, out proj.

Sharding (per the mp hint): 8 cores, core c owns q heads [4c..4c+3] and
kv head c.  Each core computes its 4 heads' attention output and a
partial output projection (wo rows [256c..256c+256)); the host sums the
8 partial projections (the mp "all-reduce" of row-parallel wo).

Device dataflow per core (all matmuls fp32r on the PE at 1 cyc/row):
  - qkv proj: lhsT = X^T chunks [128d,128s] (streamed), rhs = [wq|wk|wv]
    shard [128d, 384] -> psum [128 s, 384] per s-block.
  - RoPE applied in natural [s, hd] layout on DVE straight out of PSUM,
    then PE-transposed to qT [256, S] / kT [64, S] (dh on partitions).
  - v evicted natural [s, 64] + ones column -> v_aug [128, 65] per block.
  - scoresT[k,q] blocks: lhsT=kT block, rhs=qT slice; exp on ScalarE
    (scale=1/8 folded) psum->sbuf; causal mask via gpsimd.affine_select
    on diagonal blocks only; blocks above the diagonal are skipped.
  - PV: psum_o[65, 512] += v_aug.T @ p  (ones row accumulates the
    softmax denominator); normalize with DVE after partition_broadcast
    of the reciprocal denominator row.
  - out proj: out[s-block, :] partial = AT.T @ wo_shard.
"""

import math
from contextlib import ExitStack

import numpy as np

import concourse.bass as bass
import concourse.mybir as mybir
import concourse.tile as tile
from concourse import bacc
from concourse.masks import make_identity
from concourse.bass_utils import run_bass_kernel_spmd

F32 = mybir.dt.float32
F32R = mybir.dt.float32r
EXP = mybir.ActivationFunctionType.Exp

HEAD_DIM = 64
N_HEADS = 32
N_KV = 8
THETA = 500000.0
MAX_POS = 4096
N_CORES = 8

S = 2048
D = 2048
HPC = N_HEADS // N_CORES      # q heads per core = 4
QW = HPC * HEAD_DIM           # q cols per core = 256
SB = S // 128                 # 16 s-blocks of 128
DB = D // 128                 # 16 d-slices of 128
QT = S // 512                 # 4 q-tiles of 512
KB = S // 128                 # 16 k-blocks of 128


def _rope_tables(head_dim, end, theta, use_scaled=True):
    freqs = 1.0 / (theta ** (np.arange(0, head_dim, 2, dtype=np.float32) / head_dim))
    if use_scaled:  # llama3 rope scaling
        scale_factor, low_f, high_f, old_ctx = 8.0, 1.0, 4.0, 8192.0
        low_wl, high_wl = old_ctx / low_f, old_ctx / high_f
        wavelen = 2.0 * np.pi / freqs
        smooth = (old_ctx / wavelen - low_f) / (high_f - low_f)
        freqs = np.where(
            wavelen < high_wl, freqs,
            np.where(wavelen > low_wl, freqs / scale_factor,
                     (1.0 - smooth) * freqs / scale_factor + smooth * freqs),
        ).astype(np.float32)
    t = np.arange(end, dtype=np.float32)
    ang = np.outer(t, freqs)  # [end, head_dim//2]
    return np.cos(ang).astype(np.float32), np.sin(ang).astype(np.float32)


def build_kernel():
    # Bacc (not plain Bass): its compile() pass pipeline legalizes multi-wait
    # instructions (TRN2 allows at most 1 sync wait per instruction).
    nc = bacc.Bacc()
    xt = nc.declare_dram_parameter("xt", [D, S], F32R, isOutput=False)
    w_all = nc.declare_dram_parameter("w_all", [D, QW + 2 * HEAD_DIM], F32R, isOutput=False)
    wo = nc.declare_dram_parameter("wo", [QW, D], F32R, isOutput=False)
    cos_d = nc.declare_dram_parameter("cos", [S, HEAD_DIM // 2], F32, isOutput=False)
    sin_d = nc.declare_dram_parameter("sin", [S, HEAD_DIM // 2], F32, isOutput=False)
    out_d = nc.declare_dram_parameter("out", [S, D], F32, isOutput=True)

    with ExitStack() as ctx:
        tc = ctx.enter_context(tile.TileContext(nc))

        # ---- persistent sbuf tensors (bufs=1, distinct tags) ----
        persist = ctx.enter_context(tc.tile_pool(name="persist", bufs=1))
        w_sb = [persist.tile([128, 384], F32R, tag=f"w{d}", name=f"w{d}") for d in range(DB)]
        cos_sb = persist.tile([128, SB * 32], F32, tag="cos", name="cos_sb")   # block m at cols 32m
        sin_sb = persist.tile([128, SB * 32], F32, tag="sin", name="sin_sb")
        qT_sb = [persist.tile([128, S], F32R, tag=f"qT{t}", name=f"qT{t}") for t in range(2)]
        # kT duplicated into both partition halves so scores lhsT can match
        # the q head's partition base (matmul requires equal base partitions)
        kT_sb = persist.tile([128, S], F32R, tag="kT", name="kT")
        v_aug = [persist.tile([128, HEAD_DIM + 1], F32R, tag=f"vaug{m}", name=f"vaug{m}") for m in range(SB)]
        AT_sb = [persist.tile([128, S], F32R, tag=f"AT{t}", name=f"AT{t}") for t in range(2)]
        wo_sb = [persist.tile([128, D], F32R, tag=f"wo{t}", name=f"wo{t}") for t in range(2)]
        ident = persist.tile([128, 128], F32, tag="ident")

        # ---- rotating pools ----
        xt_pool = ctx.enter_context(tc.tile_pool(name="xt", bufs=2))
        rot_pool = ctx.enter_context(tc.tile_pool(name="rot", bufs=3))
        tmp_pool = ctx.enter_context(tc.tile_pool(name="tmp", bufs=4))
        p_pool = ctx.enter_context(tc.tile_pool(name="p", bufs=4))
        o_sb_pool = ctx.enter_context(tc.tile_pool(name="osb", bufs=2))
        nrm_pool = ctx.enter_context(tc.tile_pool(name="nrm", bufs=3))
        ps_mm = ctx.enter_context(tc.tile_pool(name="psmm", bufs=2, space="PSUM"))
        ps_s = ctx.enter_context(tc.tile_pool(name="pss", bufs=2, space="PSUM"))
        ps_o = ctx.enter_context(tc.tile_pool(name="pso", bufs=2, space="PSUM"))

        # ---- setup: weights, tables, identity, ones columns ----
        make_identity(nc, ident)
        for d in range(DB):
            nc.sync.dma_start(out=w_sb[d], in_=w_all[d * 128:(d + 1) * 128, :])
        # cos/sin: DRAM [S, 32] -> sbuf [128, (m f)] ; partition p = s % 128
        cos_r = cos_d.rearrange("(m p) f -> p m f", p=128)
        sin_r = sin_d.rearrange("(m p) f -> p m f", p=128)
        nc.sync.dma_start(out=cos_sb.rearrange("p (m f) -> p m f", m=SB), in_=cos_r)
        nc.sync.dma_start(out=sin_sb.rearrange("p (m f) -> p m f", m=SB), in_=sin_r)
        for t in range(2):
            nc.sync.dma_start(out=wo_sb[t], in_=wo[t * 128:(t + 1) * 128, :])
        ones_col = persist.tile([128, 1], F32, tag="ones", name="ones_col")
        nc.vector.memset(ones_col, 1.0)
        # sel_bc[t][p_in, p_out] = 1 iff p_in == 64t + 32*(p_out//64):
        # lhsT for the denominator-broadcast matmuls
        sel_bc = [persist.tile([128, 128], F32R, tag=f"sel{t}", name=f"sel{t}")
                  for t in range(2)]
        for t in range(2):
            zeros_f32 = nrm_pool.tile([128, 128], F32, tag="rdb")
            nc.vector.memset(zeros_f32, 0.0)
            nc.vector.tensor_copy(sel_bc[t], zeros_f32)
            nc.vector.tensor_copy(
                sel_bc[t][64 * t:64 * t + 1, 0:64],
                ones_col.broadcast_to([128, 64])[64 * t:64 * t + 1, :])
            nc.vector.tensor_copy(
                sel_bc[t][64 * t + 32:64 * t + 33, 64:128],
                ones_col.broadcast_to([128, 128])[64 * t + 32:64 * t + 33, 64:128])
        for m in range(SB):
            # memset can't write f32r; copy-cast from an f32 ones tile
            nc.vector.tensor_copy(v_aug[m][:, HEAD_DIM:HEAD_DIM + 1], ones_col)

        # ---- phase B: qkv projection + rope + transposes ----
        xt_r = xt.rearrange("(db p) s -> p db s", p=128)
        for m in range(SB):
            # one batched DMA for the whole X^T column block of this s-tile
            xcol = xt_pool.tile([128, DB * 128], F32R, tag="xcol")
            nc.sync.dma_start(
                out=xcol.rearrange("p (db c) -> p db c", db=DB),
                in_=xt_r[:, :, m * 128:(m + 1) * 128])
            ps_qkv = ps_mm.tile([128, 512], F32, tag="mm")
            for d in range(DB):
                nc.tensor.matmul(
                    ps_qkv[:, 0:384],
                    lhsT=xcol[:, d * 128:(d + 1) * 128],
                    rhs=w_sb[d],
                    start=(d == 0), stop=(d == DB - 1))

            cm = cos_sb[:, m * 32:(m + 1) * 32]
            sm = sin_sb[:, m * 32:(m + 1) * 32]
            # fused rope over q (4 heads) and k (1 head): cols 0..319 have a
            # uniform [5, 32, 2] (group, freq, re/im) structure
            cm5 = cm[:, None, :].broadcast_to([128, HPC + 1, 32])
            sm5 = sm[:, None, :].broadcast_to([128, HPC + 1, 32])
            qk5 = ps_qkv[:, 0:QW + HEAD_DIM].rearrange(
                "p (g f t) -> p g f t", g=HPC + 1, t=2)
            qk_re, qk_im = qk5[:, :, :, 0], qk5[:, :, :, 1]
            qkrot = rot_pool.tile([128, QW + HEAD_DIM], F32, tag="qkrot")
            qkr5 = qkrot.rearrange("p (g f t) -> p g f t", g=HPC + 1, t=2)
            e1 = tmp_pool.tile([128, HPC + 1, 32], F32, tag="t1")
            e2 = tmp_pool.tile([128, HPC + 1, 32], F32, tag="t2")
            nc.vector.tensor_mul(e1, qk_re, cm5)
            nc.vector.tensor_mul(e2, qk_im, sm5)
            nc.vector.tensor_sub(qkr5[:, :, :, 0], e1, e2)
            o1 = tmp_pool.tile([128, HPC + 1, 32], F32, tag="t3")
            o2 = tmp_pool.tile([128, HPC + 1, 32], F32, tag="t4")
            nc.vector.tensor_mul(o1, qk_re, sm5)
            nc.vector.tensor_mul(o2, qk_im, cm5)
            nc.vector.tensor_add(qkr5[:, :, :, 1], o1, o2)

            # v: straight eviction (no rope), natural [s, 64]
            nc.vector.tensor_copy(
                v_aug[m][:, 0:HEAD_DIM], ps_qkv[:, QW + HEAD_DIM:QW + 2 * HEAD_DIM])

            # transposes -> qT, kT
            for t in range(2):
                pst = ps_mm.tile([128, 512], F32, tag="mm")
                nc.tensor.transpose(
                    pst[:, 0:128], qkrot[:, t * 128:(t + 1) * 128], ident)
                nc.vector.tensor_copy(
                    qT_sb[t][:, m * 128:(m + 1) * 128], pst[:, 0:128])
            psk = ps_mm.tile([128, 512], F32, tag="mm")
            nc.tensor.transpose(psk[0:64, 0:128], qkrot[:, QW:QW + HEAD_DIM], ident)
            nc.vector.tensor_copy(
                kT_sb[0:64, m * 128:(m + 1) * 128], psk[0:64, 0:128])
            nc.vector.tensor_copy(
                kT_sb[64:128, m * 128:(m + 1) * 128], psk[0:64, 0:128])

        # ---- phase C: attention (j outer for early out-proj), phase D folded in ----
        for j in range(QT):
            den4 = nrm_pool.tile([128, 512], F32, tag="den4")
            nc.vector.memset(den4, 1.0)
            for h in range(HPC):
                pso = ps_o.tile([HEAD_DIM + 1, 512], F32, tag="o")
                ilast = 4 * j + 3
                qslice = qT_sb[h // 2][64 * (h % 2):64 * (h % 2) + 64,
                                      j * 512:(j + 1) * 512]
                for i0 in range(0, 4 * j + 4, 2):
                    # two k-blocks share one 2-bank psum tile so the exp
                    # (and diagonal mask) runs once per 1024 columns,
                    # amortizing the fixed per-instruction overhead
                    pss = ps_s.tile([128, 1024], F32, tag="s")
                    for u in range(2):
                        nc.tensor.matmul(
                            pss[:, u * 512:(u + 1) * 512],
                            lhsT=kT_sb[64 * (h % 2):64 * (h % 2) + 64,
                                       (i0 + u) * 128:(i0 + u) * 128 + 128],
                            rhs=qslice,
                            start=True, stop=True)
                    p = p_pool.tile([128, 1024], F32R, tag="p")
                    nc.scalar.activation(p, pss, EXP, scale=0.125)
                    if i0 + 1 >= 4 * j:  # pair touches the diagonal band
                        # keep where (512j + qq) - 128(i0+u) - kk >= 0
                        nc.gpsimd.affine_select(
                            out=p.rearrange("p (u q) -> p u q", u=2),
                            in_=p.rearrange("p (u q) -> p u q", u=2),
                            compare_op=mybir.AluOpType.is_ge,
                            fill=0.0,
                            base=512 * j - 128 * i0,
                            channel_multiplier=-1,
                            pattern=[[-128, 2], [1, 512]])
                    for u in range(2):
                        nc.tensor.matmul(
                            pso,
                            lhsT=v_aug[i0 + u],
                            rhs=p[:, u * 512:(u + 1) * 512],
                            start=(i0 + u == 0), stop=(i0 + u == ilast))
                # evict unnormalized output + denom row now (frees pso)
                nc.vector.tensor_copy(
                    AT_sb[h // 2][64 * (h % 2):64 * (h % 2) + 64,
                                  j * 512:(j + 1) * 512],
                    pso[0:HEAD_DIM, :])
                nc.vector.tensor_copy(
                    den4[32 * h:32 * h + 1, :], pso[HEAD_DIM:HEAD_DIM + 1, :])

            # batched softmax denominators: one reciprocal for all 4 heads,
            # then per-head broadcast + in-place normalize in SBUF
            rec4 = nrm_pool.tile([128, 512], F32R, tag="rec4")
            with nc.allow_low_precision(reason="f32r recip feeds f32r bcast matmul"):
                nc.vector.reciprocal(rec4, den4)
            for t in range(2):
                # selection-matrix matmul broadcasts two heads' reciprocal
                # rows (partitions 64t, 64t+32) to partitions 0-63 / 64-127
                rdb = ps_mm.tile([128, 512], F32, tag="mm")
                nc.tensor.matmul(
                    rdb, lhsT=sel_bc[t], rhs=rec4, start=True, stop=True)
                at0 = AT_sb[t][0:64, j * 512:(j + 1) * 512]
                nc.vector.tensor_mul(at0, at0, rdb[0:64, :])
                at1 = AT_sb[t][64:128, j * 512:(j + 1) * 512]
                nc.vector.tensor_mul(at1, at1, rdb[64:128, :])

            # ---- phase D for the s-blocks covered by this q-tile ----
            for m in range(4 * j, 4 * j + 4):
                out_sb = o_sb_pool.tile([128, D], F32, tag="out")
                for n in range(4):
                    psd = ps_mm.tile([128, 512], F32, tag="mm")
                    for t in range(2):
                        nc.tensor.matmul(
                            psd,
                            lhsT=AT_sb[t][:, m * 128:(m + 1) * 128],
                            rhs=wo_sb[t][:, n * 512:(n + 1) * 512],
                            start=(t == 0), stop=(t == 1))
                    if n % 2 == 0:
                        nc.scalar.copy(out_sb[:, n * 512:(n + 1) * 512], psd)
                    else:
                        nc.vector.tensor_copy(out_sb[:, n * 512:(n + 1) * 512], psd)
                nc.sync.dma_start(
                    out=out_d[m * 128:(m + 1) * 128, :], in_=out_sb)

    nc.finalize()
    return nc


_NC_CACHE = None


def _get_nc():
    global _NC_CACHE
    if _NC_CACHE is None:
        _NC_CACHE = build_kernel()
    return _NC_CACHE


def kernel(hidden_states, attention_mask, position_ids, wq, wk, wv, wo,
           _trace=False, _trace_kwargs=None):
    hidden_states = np.asarray(hidden_states)
    attention_mask = np.asarray(attention_mask)
    position_ids = np.asarray(position_ids)
    wq, wk, wv, wo = (np.asarray(a) for a in (wq, wk, wv, wo))
    B, S_, D_ = hidden_states.shape
    assert (B, S_, D_) == (1, S, D)
    out_dtype = hidden_states.dtype

    seg = attention_mask[0]
    if not np.all(seg == seg[0]):
        raise NotImplementedError(
            "non-uniform attention_mask (multi-segment) not supported")

    cos_t, sin_t = _rope_tables(HEAD_DIM, MAX_POS, THETA, use_scaled=True)
    pos = position_ids[0].astype(np.int64)
    cos = np.ascontiguousarray(cos_t[pos])  # [S, 32]
    sin = np.ascontiguousarray(sin_t[pos])

    x = np.ascontiguousarray(hidden_states[0].astype(np.float32))
    xt = np.ascontiguousarray(x.T)  # [D, S]

    in_maps = []
    for c in range(N_CORES):
        w_cat = np.ascontiguousarray(np.concatenate([
            wq[:, c * QW:(c + 1) * QW],
            wk[:, c * HEAD_DIM:(c + 1) * HEAD_DIM],
            wv[:, c * HEAD_DIM:(c + 1) * HEAD_DIM],
        ], axis=1).astype(np.float32))
        wo_rows = np.ascontiguousarray(
            wo[c * QW:(c + 1) * QW, :].astype(np.float32))
        in_maps.append({
            "xt": xt, "w_all": w_cat, "wo": wo_rows, "cos": cos, "sin": sin,
        })

    nc = _get_nc()
    kw = {}
    if _trace:
        kw["trace"] = True
        if _trace_kwargs:
            kw.update(_trace_kwargs)
    res = run_bass_kernel_spmd(nc, in_maps, list(range(N_CORES)), **kw)
    out = np.zeros((S, D), np.float32)
    for c in range(N_CORES):
        out += res.results[c]["out"]
    kernel._last_result = res
    return out.reshape(1, S, D).astype(out_dtype)
